# revision 5
# baseline (speedup 1.0000x reference)
"""Multi-head attention (RoPE + softmax) forward for Trainium2, 8 NeuronCores.

v5 over v4:
  - stage-2 wide tiles pair TWO QC CHUNKS OF THE SAME KT ([128, 1024] =
    scores(kt, qc0)|scores(kt, qc1)), so the exp bias is uniform per tile and
    the mask goes back into the activation bias — em weighting machinery
    (v-scaling, scaled folds) is gone; denominator fold is plain bf16 adds.
  - denominator cross-partition sum via gpsimd partition_all_reduce (Pool
    engine), reciprocal in place on DVE: no PSUM banks for softmax stats;
    PSUM = ps1(2) + pss(2x2) + pso(1x2) = 8 banks with projections live.
  - w DMAs split for earlier first-chain start; x chunks even-first.

All matmul operands bf16, PSUM fp32. Layouts: see v2/v3 docstrings.
"""

import math
from contextlib import ExitStack

import numpy as np

B, S, D = 4, 2048, 2048
H_PER_CORE = 8
HD = 128
F = 1024
P = 128
DT = D // P  # 16
NCORES = 8
SCALE = 1.0 / math.sqrt(HD)

_CACHE = {}


def _build():
    import concourse.bacc as bacc
    import concourse.bass_isa as bass_isa
    import concourse.mybir as mybir
    import concourse.tile as tile

    f32 = mybir.dt.float32
    bf16 = mybir.dt.bfloat16
    EXP = mybir.ActivationFunctionType.Exp
    ADD = mybir.AluOpType.add

    nc = bacc.Bacc("TRN2", target_bir_lowering=False, debug=False, num_devices=NCORES)

    xh_d = nc.dram_tensor("xh", [P, DT * S], bf16, kind="ExternalInput")
    wq_d = nc.dram_tensor("wqh", [H_PER_CORE * P, DT * P], bf16, kind="ExternalInput")
    wk_d = nc.dram_tensor("wkh", [H_PER_CORE * P, DT * P], bf16, kind="ExternalInput")
    wv_d = nc.dram_tensor("wvh", [2 * P, DT * 512], bf16, kind="ExternalInput")
    wo_d = nc.dram_tensor("woh", [P, H_PER_CORE * S], bf16, kind="ExternalInput")
    cos_d = nc.dram_tensor("cosT", [P, S], bf16, kind="ExternalInput")
    sin_d = nc.dram_tensor("sinT", [P, S], bf16, kind="ExternalInput")
    mask_d = nc.dram_tensor("maskT", [P, DT], f32, kind="ExternalInput")
    out_d = nc.dram_tensor("out", [S, D], f32, kind="ExternalOutput")
    attn_d = nc.dram_tensor("attn_scratch", [H_PER_CORE * P, S], bf16, kind="Internal")

    with tile.TileContext(nc) as tc, nc.allow_low_precision(
        reason="bf16 operands feeding bf16 matmuls; PSUM accumulation stays fp32"
    ):
        with ExitStack() as outer:
            constp = outer.enter_context(tc.tile_pool(name="const", bufs=1))
            kresp = outer.enter_context(tc.tile_pool(name="kres", bufs=1))
            vresp = outer.enter_context(tc.tile_pool(name="vres", bufs=1))
            qringp = outer.enter_context(tc.tile_pool(name="qring", bufs=3))
            expp = outer.enter_context(tc.tile_pool(name="exp2", bufs=2))
            eaccp = outer.enter_context(tc.tile_pool(name="eacc", bufs=2))
            dallp = outer.enter_context(tc.tile_pool(name="dall", bufs=2))
            atilep = outer.enter_context(tc.tile_pool(name="atile", bufs=3))

            mask_sb = constp.tile([P, DT], f32, name="mask_sb")
            nc.sync.dma_start(out=mask_sb[:], in_=mask_d[:])

            kres = [kresp.tile([P, S], bf16, name=f"kres{h}") for h in range(H_PER_CORE)]
            vres = [vresp.tile([P, F], bf16, name=f"vres{st}") for st in range(DT)]

            with ExitStack() as s1:
                cossinp = s1.enter_context(tc.tile_pool(name="cossin", bufs=1))
                xpool = s1.enter_context(tc.tile_pool(name="xpool", bufs=1))
                wpool = s1.enter_context(tc.tile_pool(name="wpool", bufs=2))
                rotp = s1.enter_context(tc.tile_pool(name="rot", bufs=4))
                ps1 = s1.enter_context(tc.tile_pool(name="ps1", bufs=2, space="PSUM"))
                pss_pool = s1.enter_context(
                    tc.tile_pool(name="ps_s", bufs=2, space="PSUM")
                )
                pso_pool = s1.enter_context(
                    tc.tile_pool(name="ps_o", bufs=1, space="PSUM")
                )
                cos_sb = cossinp.tile([P, S], bf16, name="cos_sb")
                sin_sb = cossinp.tile([P, S], bf16, name="sin_sb")
                nc.sync.dma_start(out=cos_sb[:], in_=cos_d[:])
                nc.sync.dma_start(out=sin_sb[:], in_=sin_d[:])

                x_sb = xpool.tile([P, DT * S], bf16, name="x_sb")
                for j, ch in enumerate(
                    [2 * i for i in range(16)] + [2 * i + 1 for i in range(16)]
                ):
                    eng = nc.sync if j % 2 == 0 else nc.scalar
                    eng.dma_start(
                        out=x_sb[:, ch * 1024 : (ch + 1) * 1024],
                        in_=xh_d[:, ch * 1024 : (ch + 1) * 1024],
                    )

                # ---- v pass (natural [s, f] layout, fc halves of 512) ----
                with tc.tile_pool(name="wvpool", bufs=1) as wvpool:
                    for fc in range(2):
                        wv_sb = wvpool.tile(
                            [P, DT * 512], bf16, tag="wv", name=f"wv{fc}"
                        )
                        for j in range(4):
                            nc.sync.dma_start(
                                out=wv_sb[:, j * 2048 : (j + 1) * 2048],
                                in_=wv_d[fc * P : (fc + 1) * P, j * 2048 : (j + 1) * 2048],
                            )
                        for st in range(DT):
                            ps = ps1.tile([P, 512], f32, tag="ps", name="psv")
                            for dt in range(DT):
                                nc.tensor.matmul(
                                    ps[:],
                                    lhsT=x_sb[:, dt * S + st * P : dt * S + (st + 1) * P],
                                    rhs=wv_sb[:, dt * 512 : (dt + 1) * 512],
                                    start=(dt == 0),
                                    stop=(dt == DT - 1),
                                )
                            nc.scalar.copy(vres[st][:, fc * 512 : (fc + 1) * 512], ps[:])

                def proj_head(wdram, ft, dest):
                    """dest [P, S] <- rotary(w_ft.T @ x)."""
                    w_sb = wpool.tile([P, DT * P], bf16, tag="wqk", name=f"w{ft}")
                    for j in range(4):
                        nc.sync.dma_start(
                            out=w_sb[:, j * 512 : (j + 1) * 512],
                            in_=wdram[ft * P : (ft + 1) * P, j * 512 : (j + 1) * 512],
                        )
                    for sc in range(4):
                        ps = ps1.tile([P, 512], f32, tag="ps", name="psp")
                        for dt in range(DT):
                            nc.tensor.matmul(
                                ps[:],
                                lhsT=w_sb[:, dt * P : (dt + 1) * P],
                                rhs=x_sb[:, dt * S + sc * 512 : dt * S + (sc + 1) * 512],
                                start=(dt == 0),
                                stop=(dt == DT - 1),
                            )
                        cs = cos_sb[:, sc * 512 : (sc + 1) * 512]
                        sn = sin_sb[:, sc * 512 : (sc + 1) * 512]
                        t2 = rotp.tile([P, 512], bf16, tag="t2", name="t2")
                        m1 = rotp.tile([P, 512], bf16, tag="m1", name="m1")
                        nc.vector.tensor_mul(t2[0:64, :], ps[64:128, :], sn[0:64, :])
                        nc.vector.tensor_mul(t2[64:128, :], ps[0:64, :], sn[64:128, :])
                        nc.vector.tensor_mul(m1[:], ps[:], cs)
                        nc.vector.tensor_add(
                            dest[:, sc * 512 : (sc + 1) * 512], m1[:], t2[:]
                        )

                # ---- k pass ----
                for ft in range(H_PER_CORE):
                    proj_head(wk_d, ft, kres[ft])

                # ---- q(0), then attention(h) interleaved with q(h+1) ----
                qtiles = {}
                qtiles[0] = qringp.tile([P, S], bf16, tag="q", name="q0")
                proj_head(wq_d, 0, qtiles[0])

                for h in range(H_PER_CORE):
                    q_sb = qtiles[h]
                    for qp in range(2):  # qc pairs: columns (2*qp, 2*qp+1)
                        c0 = qp * 1024
                        pso = pso_pool.tile([P, 1024], f32, tag="pso", name="pso")
                        e_acc = eaccp.tile([P, 1024], bf16, tag="ea", name="e_acc")
                        for kt in range(DT):
                            pss = pss_pool.tile([P, 1024], f32, tag="pss", name="pss")
                            for i in range(2):
                                nc.tensor.matmul(
                                    pss[:, i * 512 : (i + 1) * 512],
                                    lhsT=kres[h][:, kt * P : (kt + 1) * P],
                                    rhs=q_sb[:, c0 + i * 512 : c0 + (i + 1) * 512],
                                    start=True,
                                    stop=True,
                                )
                            e_sb = expp.tile([P, 1024], bf16, tag="e", name="e_sb")
                            nc.scalar.activation(
                                e_sb[:],
                                pss[:],
                                EXP,
                                bias=mask_sb[:, kt : kt + 1],
                                scale=SCALE,
                            )
                            if kt == 0:
                                nc.vector.tensor_copy(e_acc[:], e_sb[:])
                            else:
                                nc.vector.tensor_add(e_acc[:], e_acc[:], e_sb[:])
                            for i in range(2):
                                nc.tensor.matmul(
                                    pso[:, i * 512 : (i + 1) * 512],
                                    lhsT=vres[kt][:, h * P : (h + 1) * P],
                                    rhs=e_sb[:, i * 512 : (i + 1) * 512],
                                    start=(kt == 0),
                                    stop=(kt == DT - 1),
                                )
                        d_all = dallp.tile([P, 1024], f32, tag="d", name="d_all")
                        nc.gpsimd.partition_all_reduce(
                            d_all[:], e_acc[:], channels=P, reduce_op=bass_isa.ReduceOp.add
                        )
                        nc.vector.reciprocal(d_all[:], d_all[:])
                        a_sb = atilep.tile([P, 1024], bf16, tag="a", name="a_sb")
                        nc.vector.tensor_mul(a_sb[:], pso[:], d_all[:])
                        nc.sync.dma_start(
                            out=attn_d[h * P : (h + 1) * P, c0 : c0 + 1024],
                            in_=a_sb[:],
                        )
                    if h + 1 < H_PER_CORE:
                        qtiles[h + 1] = qringp.tile([P, S], bf16, tag="q", name=f"q{h+1}")
                        proj_head(wq_d, h + 1, qtiles[h + 1])

            # ---- Stage 3: reload attn, output projection ----
            with ExitStack() as s3:
                aresp = s3.enter_context(tc.tile_pool(name="aresb", bufs=1))
                wop = s3.enter_context(tc.tile_pool(name="wo3", bufs=1))
                osp = s3.enter_context(tc.tile_pool(name="os3", bufs=6))
                ps3 = s3.enter_context(tc.tile_pool(name="ps3", bufs=3, space="PSUM"))
                wo_sb = wop.tile([P, H_PER_CORE * S], bf16, name="wo_sb")
                for i in range(4):
                    nc.scalar.dma_start(
                        out=wo_sb[:, i * 4096 : (i + 1) * 4096],
                        in_=wo_d[:, i * 4096 : (i + 1) * 4096],
                    )
                ares = [
                    aresp.tile([P, S], bf16, name=f"ares{h}") for h in range(H_PER_CORE)
                ]
                for h in range(H_PER_CORE):
                    nc.scalar.dma_start(
                        out=ares[h][:], in_=attn_d[h * P : (h + 1) * P, :]
                    )
                for st in range(DT):
                    for ec in range(4):
                        ps = ps3.tile([P, 512], f32, tag="ps3", name="ps3t")
                        for h in range(H_PER_CORE):
                            nc.tensor.matmul(
                                ps[:],
                                lhsT=ares[h][:, st * P : (st + 1) * P],
                                rhs=wo_sb[:, h * S + ec * 512 : h * S + (ec + 1) * 512],
                                start=(h == 0),
                                stop=(h == H_PER_CORE - 1),
                            )
                        o_sb = osp.tile([P, 512], f32, tag="o3", name="o3")
                        nc.scalar.copy(o_sb[:], ps[:])
                        nc.sync.dma_start(
                            out=out_d[st * P : (st + 1) * P, ec * 512 : (ec + 1) * 512],
                            in_=o_sb[:],
                        )

    nc.compile()
    return nc


def _host_prep(inputs):
    import ml_dtypes

    bf = ml_dtypes.bfloat16
    x = np.asarray(inputs["x"], np.float32)
    fc = np.asarray(inputs["freqs_cos"], np.float32)
    fs = np.asarray(inputs["freqs_sin"], np.float32)
    mask = np.asarray(inputs["mask"], np.float32)
    wq = np.asarray(inputs["wq"], np.float32)
    wk = np.asarray(inputs["wk"], np.float32)
    wv = np.asarray(inputs["wv"], np.float32)
    wo = np.asarray(inputs["wo"], np.float32)

    perm = np.concatenate([np.arange(0, HD, 2), np.arange(1, HD, 2)])
    cosT = np.ascontiguousarray(np.concatenate([fc.T, fc.T], 0)).astype(bf)
    sinT = np.ascontiguousarray(np.concatenate([-fs.T, fs.T], 0)).astype(bf)

    in_maps = []
    for c in range(NCORES):
        b, g = c // 2, c % 2
        xh = np.ascontiguousarray(
            x[b].T.reshape(DT, P, S).transpose(1, 0, 2).reshape(P, DT * S)
        ).astype(bf)

        def wqk_layout(w):
            cols = w[:, g * F : (g + 1) * F]
            cols = cols.reshape(D, H_PER_CORE, HD)[:, :, perm]
            arr = cols.reshape(DT, P, H_PER_CORE, HD).transpose(2, 1, 0, 3)
            return np.ascontiguousarray(arr.reshape(H_PER_CORE * P, DT * P)).astype(bf)

        vcols = wv[:, g * F : (g + 1) * F].reshape(DT, P, 2, 512).transpose(2, 1, 0, 3)
        wvh = np.ascontiguousarray(vcols.reshape(2 * P, DT * 512)).astype(bf)
        woh = np.ascontiguousarray(
            wo[g * F : (g + 1) * F, :].reshape(H_PER_CORE, P, S).transpose(1, 0, 2).reshape(P, H_PER_CORE * S)
        ).astype(bf)
        in_maps.append(
            {
                "xh": xh,
                "wqh": wqk_layout(wq),
                "wkh": wqk_layout(wk),
                "wvh": wvh,
                "woh": woh,
                "cosT": cosT,
                "sinT": sinT,
                "maskT": np.ascontiguousarray(mask[b].reshape(DT, P).T),
            }
        )
    return in_maps


def kernel(**inputs):
    from concourse.bass_utils import run_bass_kernel_spmd

    if "nc" not in _CACHE:
        _CACHE["nc"] = _build()
    nc = _CACHE["nc"]

    in_maps = _host_prep(inputs)
    res = run_bass_kernel_spmd(nc, in_maps, core_ids=list(range(NCORES)))
    out = np.empty((B, S, D), np.float32)
    for b in range(B):
        out[b] = res.results[2 * b]["out"] + res.results[2 * b + 1]["out"]
    return out
